# revision 33
# baseline (speedup 1.0000x reference)
"""Data-dependent ALiBi bias kernel for Trainium2, distributed over 8 NeuronCores.

Reference computation (per full input):
    logits = einsum('bnd,hd->bhn', x, W) + b          # [2, 16, 2048]
    fg     = log_sigmoid(logits)                      # [2, 16, 2048]
    fg     = cumsum(fg, axis=-1)
    out    = fg[:, :, :, None] - fg[:, :, None, :]    # [2, 16, 2048, 2048]

Sharding: 32 (batch, head) pairs / 8 cores = 4 heads per core, batch-major.
Each core computes its own [4, 2048, 2048] slab independently; no collectives.

v12 design (fp8 e3m4 output, DRAM-bounce broadcast, hybrid tile widths):
  - Output dtype float8 E3M4 (4 mantissa bits, max normal 15.5) with a
    power-of-2 scale of 0.5 baked in device-side (|0.5*out| <= 12.1);
    host multiplies the decoded fp32 by 2.  rel_fro ~1.37e-2 (gate 2e-2).
    Halves output DMA bytes vs fp16.
  - Inputs fp8 E4M3 (W pre-scaled by 16 to dodge the e4m3 subnormal
    floor; 1/16 descale folded into the ACT Exp input scale).  The whole
    g chain is fp32: PSUM matmul accum -> ACT exp/ln -> DVE carry scan.
  - Broadcast of 0.5*g across partitions goes through a DRAM bounce:
    DVE mul -> small DMA to an Internal DRAM tensor -> per-(head, half)
    DMA back with a stride-0 partition AP (each partition rereads the
    same HBM row).  Zero ACT/DVE cost, no gpsimd partition_broadcast
    (it contends with DVE's SBUF port), no PE (operands must sit at base
    partition 0).  These DMAs ride the GPSIMD SWDGE ring so their
    semaphore waits never block output-tile DMAs queued on the Sync ring
    (HWDGE sequencers process their FIFO in order).
  - Output tiles: row chunks c4 in {0,1} (rows 0:1024) are HALF-width
    [128, 4, 1024] so generation starts right after the left bcast
    (~20 us) instead of waiting for full g; chunks {2,3} are full-width
    [128, 4, 2048].  Row 512*c4 + 4p + r lives at partition p, slot r;
    4-8 KB contiguous runs per partition.  Per row-slot one elementwise
    op: DVE tensor_scalar_add (~0.66/1.19 us half/full, 2x_2P mode) or
    ACT Identity+bias (~1.04/1.89 us), greedily balanced.
  - DVE tiles stream out over the Sync HWDGE ring, ACT tiles over the
    Scalar HWDGE ring: two independent in-order DMA queues, so a slow
    ACT tile at the head of one ring never blocks finished DVE tiles on
    the other.
  - ngEO biases: 16 PE transposes of stride-4 g column slices into one
    PSUM tile, scaled -0.5 into SBUF by two tiny DVE muls (split per
    c4-pair so phase-A tiles don't wait on segment-3 transposes).

Negative results baked into this shape (measured):
  - Splitting each tile's DMA in two (finer trickle) slows ALL engine
    ops ~20% (concurrent DMA reads of a buffer an engine is still
    writing); RPT=8 half-width tiles do the same.  Keep one DMA per
    tile buffer, issued after its last op.
  - RPT=2 full-width tiles halve the DRAM run length to 4 KB and cost
    ~2.5 engine-us of extra DMA time on 8 MB.
  - gpsimd partition_broadcast concurrent with DVE work slows DVE ops
    up to 3.5x (shared SBUF port).

Hardware gotchas baked in: PE matmul/transpose operands at base
partition 0; PSUM never a DMA source; ACT stays on one activation table
set (set 6: exp+ln+identity); fp8 E4M3/E3M4 host encodings via ml_dtypes
match the TRN bit formats for all finite in-range values.
"""

import numpy as np
import ml_dtypes

B = 2
NH = 16
N = 2048
D = 1024
NCORES = 8
HPC = (B * NH) // NCORES  # 4 (batch, head) pairs per core
P = 128
DC = D // P      # 8 contraction chunks
SW = 512         # segment width (= max matmul moving free dim)
NSEG = N // SW   # 4
RPT = 4          # rows per partition in an output tile
NCH4 = N // (P * RPT)  # 4 output row-chunks (512 rows each) per head
NH2 = N // 2

_CACHE = {}


def _build_nc():
    import concourse.bacc as bacc
    import concourse.mybir as mybir
    from concourse.masks import make_identity
    from concourse.tile import TileContext

    f32 = mybir.dt.float32
    f16 = mybir.dt.float16
    f8e4 = mybir.dt.float8e4
    f8e3 = mybir.dt.float8e3
    Act = mybir.ActivationFunctionType
    nc = bacc.Bacc(None, target_bir_lowering=False)

    # xT host-pre-arranged seg-major/partition-major:
    # xT[s, p, c, j] = e4m3(x^T[c*128+p, s*512+j])
    xT = nc.dram_tensor("xT", [NSEG, P, DC, SW], f8e4, kind="ExternalInput")
    Wt = nc.dram_tensor("Wt", [D, HPC], f8e4, kind="ExternalInput")  # 16*W
    bv = nc.dram_tensor("bv", [HPC, 1], f32, kind="ExternalInput")
    # 0.5*g bounce rows for the stride-0 broadcast reads
    gdram = nc.dram_tensor("gdram", [HPC, N], f16, kind="Internal")
    # outputs: rows 0:1024 split into column halves (half-width tiles),
    # rows 1024:2048 full-width
    outA = nc.dram_tensor("outA", [HPC, NH2, NH2], f8e3, kind="ExternalOutput")
    outB = nc.dram_tensor("outB", [HPC, NH2, NH2], f8e3, kind="ExternalOutput")
    outC = nc.dram_tensor("outC", [HPC, NH2, N], f8e3, kind="ExternalOutput")
    # view row i = c4*512 + 4p + r at [h, c4, p, r, :]
    outA_r = outA.rearrange("h (p r) n -> h p r n", p=P)
    outB_r = outB.rearrange("h (p r) n -> h p r n", p=P)
    outC_r = outC.rearrange("h (c4 p r) n -> h c4 p r n", p=P, r=RPT)

    with TileContext(nc) as tc:
        with (
            tc.tile_pool(name="big", bufs=1) as big,
            tc.tile_pool(name="small", bufs=1) as small,
            tc.tile_pool(name="useg", bufs=2) as usegp,
            tc.tile_pool(name="outa", bufs=3) as outa,
            tc.tile_pool(name="outv", bufs=4) as outv,
            tc.tile_pool(name="lpsp", bufs=3, space="PSUM") as lps,
            tc.tile_pool(name="trp", bufs=1, space="PSUM") as trp,
        ):
            # ---- inputs -> SBUF. xT seg 0 first (gates the first matmul),
            # then b (gates nb -> the first EXP), Wt (gates ldweights),
            # remaining segs.
            xT_s = big.tile([P, NSEG, DC, SW], f8e4, tag="xT")
            nc.sync.dma_start(out=xT_s[:, 0], in_=xT[0])
            b_s = small.tile([HPC, 1], f32, tag="b")
            nc.sync.dma_start(out=b_s, in_=bv[:])
            Wt_s = small.tile([P, DC, HPC], f8e4, tag="Wt")
            nc.sync.dma_start(out=Wt_s, in_=Wt.rearrange("(c p) h -> p c h", p=P))
            for s in range(1, NSEG):
                nc.sync.dma_start(out=xT_s[:, s], in_=xT[s])
            nb = small.tile([HPC, 1], f32, tag="nb")
            nc.vector.tensor_scalar_mul(nb, b_s, -1.0)
            # one explicit load of the combined exp+ln+identity table, issued
            # while the x^T DMA streams
            ACT_SET_LN_EXP = 6  # natural_log_exp_and_others in act_info.json
            nc.scalar.add_instruction(
                mybir.InstLoadActFuncSet(
                    name=f"I-{nc.next_id()}",
                    act_func_set_id=ACT_SET_LN_EXP,
                    engine=mybir.EngineType.Activation,
                )
            )

            ident = small.tile([HPC, HPC], f32, tag="ident")
            make_identity(nc, ident)
            zeros = small.tile([HPC, SW], f32, tag="zeros")
            nc.gpsimd.memset(zeros, 0.0)

            g = small.tile([HPC, N], f32, tag="g")
            gS16 = small.tile([HPC, N], f16, tag="gS16")
            # tr8[p, r, h] = g[h, 8p + r] rows 0:1024 (stride-8 transpose);
            # tr[p, r, c4, h] = g[h, 512*c4 + 4p + r] for c4 in {2, 3}
            tr8 = trp.tile([P, 8, HPC], f32, tag="tr8")
            tr = trp.tile([P, RPT, 2, HPC], f32, tag="tr")
            ngEO8 = small.tile([P, 8, HPC], f32, tag="ngEO8")
            ngEO = small.tile([P, RPT, 2 * HPC], f32, tag="ngEO")
            bcast16 = big.tile([P, HPC, N], f16, tag="bcast16")

            ps_tiles = {}

            def chain_mm(s):
                # (16*logits)^T [4, 512] for segment s, accumulated in PSUM
                ps = lps.tile([HPC, SW], f32, tag="lps")
                ps_tiles[s] = ps
                for c in range(DC):
                    nc.tensor.matmul(
                        ps,
                        Wt_s[:, c, :],
                        xT_s[:, s, c, :],
                        start=(c == 0),
                        stop=(c == DC - 1),
                    )

            def chain_post(s):
                lo, hi = s * SW, (s + 1) * SW
                ps = ps_tiles.pop(s)
                us = usegp.tile([HPC, SW], f32, tag="useg")
                # t = exp(-(logits + b)) = exp(ps * (-1/16) - b)
                nc.scalar.activation(us, ps, Act.Exp, bias=nb[:, 0:1], scale=-1.0 / 16.0)
                # u = ln(1 + t)  (= -logsigmoid(logits), positive)
                nc.scalar.activation(us, us, Act.Ln, bias=1.0)
                # g[:, lo:hi] = cumsum(useg) carried from the previous segment
                init = 0.0 if s == 0 else g[:, lo - 1 : lo]
                nc.vector.tensor_tensor_scan(
                    g[:, lo:hi], us, zeros, init,
                    mybir.AluOpType.add, mybir.AluOpType.add,
                )
                # transposes: rows 0:1024 stride-8 after seg 1 (RPT=8
                # half-width tiles); rows 1024:2048 stride-4 per segment
                if s == 1:
                    for r in range(8):
                        nc.tensor.transpose(
                            tr8[:, r, :], g[:, r : NH2 : 8], ident
                        )
                elif s >= 2:
                    for r in range(RPT):
                        nc.tensor.transpose(
                            tr[:, r, s - 2, :], g[:, lo + r : hi : RPT], ident
                        )

            def half_done(half):
                # gS16 half = 0.5*g half (fp16); bounce to DRAM; fan out to
                # all 128 partitions per head via stride-0 reads
                lo = half * NH2
                nc.vector.tensor_scalar_mul(
                    gS16[:, lo : lo + NH2], g[:, lo : lo + NH2], 0.5
                )
                # biases for this half's row chunks
                if half == 0:
                    nc.vector.tensor_scalar_mul(ngEO8, tr8, -0.5)
                else:
                    nc.vector.tensor_scalar_mul(
                        ngEO,
                        tr.rearrange("p r c4 h -> p r (c4 h)"),
                        -0.5,
                    )
                # stage + broadcast DMAs ride the GPSIMD SWDGE ring: their
                # semaphore waits (on the DVE muls) would otherwise block
                # queued output-tile DMAs on the Sync ring (the HWDGE
                # sequencer waits in FIFO order)
                nc.gpsimd.dma_start(
                    out=gdram[:, lo : lo + NH2], in_=gS16[:, lo : lo + NH2]
                )
                for h in range(HPC):
                    nc.gpsimd.dma_start(
                        out=bcast16[:, h, lo : lo + NH2],
                        in_=gdram[h : h + 1, lo : lo + NH2]
                        .partition_broadcast(P)
                        .squeeze(1),
                    )

            # software-pipelined front
            chain_mm(0)
            chain_mm(1)
            chain_post(0)
            chain_mm(2)
            chain_post(1)
            half_done(0)
            chain_mm(3)
            chain_post(2)
            chain_post(3)
            half_done(1)

            # ---- output tiles, elementwise ot = bcast16 + ngEO per row-slot,
            # greedily split between DVE (fast) and ACT; DVE tiles drain on
            # the Sync DMA ring, ACT tiles on the Scalar ring.
            eng_t = {"a": 0.0, "v": 0.0}
            COST = {("a", 0): 8 * 1.04, ("a", 1): 4 * 1.89,
                    ("v", 0): 8 * 0.66, ("v", 1): 4 * 1.19}

            def emit_tile(h, c4, half):
                # half: 0/1 = column half of rows 0:1024 (RPT=8 tiles, c4
                # unused); None = full width over row chunk c4 in {2, 3}
                full = half is None
                w = N if full else NH2
                lo = 0 if full else half * NH2
                nrow = RPT if full else 8
                use_a = (eng_t["a"] + COST[("a", int(full))]
                         < eng_t["v"] + COST[("v", int(full))])
                eng = "a" if use_a else "v"
                eng_t[eng] += COST[(eng, int(full))]
                pool = outa if use_a else outv
                ot = pool.tile([P, nrow, w], f8e3, tag=f"ot{eng}{int(full)}")
                if full:
                    dst = outC_r[h, c4 - 2]
                    bias = ngEO[:, :, (c4 - 2) * HPC + h]
                else:
                    dst = (outA_r if half == 0 else outB_r)[h]
                    bias = ngEO8[:, :, h]
                dma = (nc.scalar if use_a else nc.sync).dma_start
                for r in range(nrow):
                    if use_a:
                        nc.scalar.activation(
                            ot[:, r, :], bcast16[:, h, lo : lo + w],
                            Act.Identity,
                            bias=bias[:, r : r + 1], scale=1.0,
                        )
                    else:
                        nc.vector.tensor_scalar_add(
                            ot[:, r, :], bcast16[:, h, lo : lo + w],
                            bias[:, r : r + 1],
                        )
                dma(out=dst, in_=ot)

            # phase A: left halves of rows 0:1024; then per head the
            # full-width bottom chunks and right halves (ready after the
            # right bcast)
            for h in range(HPC):
                emit_tile(h, None, 0)
            for h in range(HPC):
                emit_tile(h, 2, None)
                emit_tile(h, 3, None)
                emit_tile(h, None, 1)

    if not nc.is_finalized():
        nc.finalize()
    return nc


def _get_nc():
    if "nc" not in _CACHE:
        _CACHE["nc"] = _build_nc()
    return _CACHE["nc"]


def _make_in_maps(x, W, b):
    x = np.ascontiguousarray(x, dtype=np.float32)
    W = np.ascontiguousarray(W, dtype=np.float32)
    b = np.ascontiguousarray(b, dtype=np.float32)
    f8e4 = ml_dtypes.float8_e4m3
    # seg-major partition-major: xT[s, p, c, j] = x[bi].T[c*128+p, s*512+j]
    xT_by_batch = [
        np.ascontiguousarray(
            x[bi].T.astype(f8e4)
            .reshape(DC, P, NSEG, SW)
            .transpose(2, 1, 0, 3)
        )
        for bi in range(B)
    ]
    in_maps = []
    for k in range(NCORES):
        bi = k // (NCORES // B)
        h0 = (k % (NCORES // B)) * HPC
        in_maps.append(
            {
                "xT": xT_by_batch[bi],
                "Wt": np.ascontiguousarray(
                    (W[h0 : h0 + HPC].T * 16.0).astype(f8e4)
                ),
                "bv": np.ascontiguousarray(b[h0 : h0 + HPC].reshape(HPC, 1)),
            }
        )
    return in_maps


def kernel(x, W, b, _trace=False, _trace_cores=None):
    from concourse.bass_utils import run_bass_kernel_spmd

    nc = _get_nc()
    in_maps = _make_in_maps(x, W, b)
    res = run_bass_kernel_spmd(
        nc, in_maps, core_ids=list(range(NCORES)), trace=_trace,
        trace_cores=_trace_cores,
    )
    _CACHE["last_results"] = res
    full = np.empty((B, NH, N, N), dtype=np.float32)
    for k in range(NCORES):
        bi = k // (NCORES // B)
        h0 = (k % (NCORES // B)) * HPC
        r = res.results[k]
        # decode e3m4; the 2x undoes the device-side 0.5 scale
        sl = full[bi, h0 : h0 + HPC]
        sl[:, :NH2, :NH2] = r["outA"].astype(np.float32)
        sl[:, :NH2, NH2:] = r["outB"].astype(np.float32)
        sl[:, NH2:, :] = r["outC"].astype(np.float32)
        np.multiply(sl, 2.0, out=sl)
    return full


# revision 34
# speedup vs baseline: 1.0271x; 1.0271x over previous
"""Data-dependent ALiBi bias kernel for Trainium2, distributed over 8 NeuronCores.

Reference computation (per full input):
    logits = einsum('bnd,hd->bhn', x, W) + b          # [2, 16, 2048]
    fg     = log_sigmoid(logits)                      # [2, 16, 2048]
    fg     = cumsum(fg, axis=-1)
    out    = fg[:, :, :, None] - fg[:, :, None, :]    # [2, 16, 2048, 2048]

Sharding: 32 (batch, head) pairs / 8 cores = 4 heads per core, batch-major.
Each core computes its own [4, 2048, 2048] slab independently; no collectives.

v12 design (fp8 e3m4 output, DRAM-bounce broadcast, hybrid tile widths):
  - Output dtype float8 E3M4 (4 mantissa bits, max normal 15.5) with a
    power-of-2 scale of 0.5 baked in device-side (|0.5*out| <= 12.1);
    host multiplies the decoded fp32 by 2.  rel_fro ~1.37e-2 (gate 2e-2).
    Halves output DMA bytes vs fp16.
  - Inputs fp8 E4M3 (W pre-scaled by 16 to dodge the e4m3 subnormal
    floor; 1/16 descale folded into the ACT Exp input scale).  The whole
    g chain is fp32: PSUM matmul accum -> ACT exp/ln -> DVE carry scan.
  - Broadcast of 0.5*g across partitions goes through a DRAM bounce:
    DVE mul -> small DMA to an Internal DRAM tensor -> per-(head, half)
    DMA back with a stride-0 partition AP (each partition rereads the
    same HBM row).  Zero ACT/DVE cost, no gpsimd partition_broadcast
    (it contends with DVE's SBUF port), no PE (operands must sit at base
    partition 0).  These DMAs ride the GPSIMD SWDGE ring so their
    semaphore waits never block output-tile DMAs queued on the Sync ring
    (HWDGE sequencers process their FIFO in order).
  - Output tiles: row chunks c4 in {0,1} (rows 0:1024) are HALF-width
    [128, 4, 1024] so generation starts right after the left bcast
    (~20 us) instead of waiting for full g; chunks {2,3} are full-width
    [128, 4, 2048].  Row 512*c4 + 4p + r lives at partition p, slot r;
    4-8 KB contiguous runs per partition.  Per row-slot one elementwise
    op: DVE tensor_scalar_add (~0.66/1.19 us half/full, 2x_2P mode) or
    ACT Identity+bias (~1.04/1.89 us), greedily balanced.
  - DVE tiles stream out over the Sync HWDGE ring, ACT tiles over the
    Scalar HWDGE ring: two independent in-order DMA queues, so a slow
    ACT tile at the head of one ring never blocks finished DVE tiles on
    the other.
  - ngEO biases: 16 PE transposes of stride-4 g column slices into one
    PSUM tile, scaled -0.5 into SBUF by two tiny DVE muls (split per
    c4-pair so phase-A tiles don't wait on segment-3 transposes).

Negative results baked into this shape (measured):
  - Splitting each tile's DMA in two (finer trickle) slows ALL engine
    ops ~20% (concurrent DMA reads of a buffer an engine is still
    writing); RPT=8 half-width tiles do the same.  Keep one DMA per
    tile buffer, issued after its last op.
  - RPT=2 full-width tiles halve the DRAM run length to 4 KB and cost
    ~2.5 engine-us of extra DMA time on 8 MB.
  - gpsimd partition_broadcast concurrent with DVE work slows DVE ops
    up to 3.5x (shared SBUF port).

Hardware gotchas baked in: PE matmul/transpose operands at base
partition 0; PSUM never a DMA source; ACT stays on one activation table
set (set 6: exp+ln+identity); fp8 E4M3/E3M4 host encodings via ml_dtypes
match the TRN bit formats for all finite in-range values.
"""

import numpy as np
import ml_dtypes

B = 2
NH = 16
N = 2048
D = 1024
NCORES = 8
HPC = (B * NH) // NCORES  # 4 (batch, head) pairs per core
P = 128
DC = D // P      # 8 contraction chunks
SW = 512         # segment width (= max matmul moving free dim)
NSEG = N // SW   # 4
RPT = 4          # rows per partition in an output tile
NCH4 = N // (P * RPT)  # 4 output row-chunks (512 rows each) per head
NH2 = N // 2

_CACHE = {}


def _build_nc():
    import concourse.bacc as bacc
    import concourse.mybir as mybir
    from concourse.masks import make_identity
    from concourse.tile import TileContext

    f32 = mybir.dt.float32
    f16 = mybir.dt.float16
    f8e4 = mybir.dt.float8e4
    f8e3 = mybir.dt.float8e3
    Act = mybir.ActivationFunctionType
    nc = bacc.Bacc(None, target_bir_lowering=False)

    # xT host-pre-arranged seg-major/partition-major:
    # xT[s, p, c, j] = e4m3(x^T[c*128+p, s*512+j])
    xT = nc.dram_tensor("xT", [NSEG, P, DC, SW], f8e4, kind="ExternalInput")
    Wt = nc.dram_tensor("Wt", [D, HPC], f8e4, kind="ExternalInput")  # 16*W
    bv = nc.dram_tensor("bv", [HPC, 1], f32, kind="ExternalInput")
    # 0.5*g bounce rows for the stride-0 broadcast reads
    gdram = nc.dram_tensor("gdram", [HPC, N], f16, kind="Internal")
    # outputs: rows 0:1024 split into column halves (half-width tiles),
    # rows 1024:2048 full-width
    outA = nc.dram_tensor("outA", [HPC, NH2, NH2], f8e3, kind="ExternalOutput")
    outB = nc.dram_tensor("outB", [HPC, NH2, NH2], f8e3, kind="ExternalOutput")
    outC = nc.dram_tensor("outC", [HPC, NH2, N], f8e3, kind="ExternalOutput")
    # view row i = c4*512 + 4p + r at [h, c4, p, r, :]
    outA_r = outA.rearrange("h (c4 p r) n -> h c4 p r n", p=P, r=RPT)
    outB_r = outB.rearrange("h (c4 p r) n -> h c4 p r n", p=P, r=RPT)
    outC_r = outC.rearrange("h (c4 p r) n -> h c4 p r n", p=P, r=RPT)

    with TileContext(nc) as tc:
        with (
            tc.tile_pool(name="big", bufs=1) as big,
            tc.tile_pool(name="small", bufs=1) as small,
            tc.tile_pool(name="useg", bufs=2) as usegp,
            tc.tile_pool(name="outa", bufs=6) as outa,
            tc.tile_pool(name="outv", bufs=7) as outv,
            tc.tile_pool(name="lpsp", bufs=3, space="PSUM") as lps,
            tc.tile_pool(name="trp", bufs=1, space="PSUM") as trp,
        ):
            # ---- inputs -> SBUF. xT seg 0 first (gates the first matmul),
            # then b (gates nb -> the first EXP), Wt (gates ldweights),
            # remaining segs.
            xT_s = big.tile([P, NSEG, DC, SW], f8e4, tag="xT")
            nc.sync.dma_start(out=xT_s[:, 0], in_=xT[0])
            b_s = small.tile([HPC, 1], f32, tag="b")
            nc.sync.dma_start(out=b_s, in_=bv[:])
            Wt_s = small.tile([P, DC, HPC], f8e4, tag="Wt")
            nc.sync.dma_start(out=Wt_s, in_=Wt.rearrange("(c p) h -> p c h", p=P))
            for s in range(1, NSEG):
                nc.sync.dma_start(out=xT_s[:, s], in_=xT[s])
            nb = small.tile([HPC, 1], f32, tag="nb")
            nc.vector.tensor_scalar_mul(nb, b_s, -1.0)
            # one explicit load of the combined exp+ln+identity table, issued
            # while the x^T DMA streams
            ACT_SET_LN_EXP = 6  # natural_log_exp_and_others in act_info.json
            nc.scalar.add_instruction(
                mybir.InstLoadActFuncSet(
                    name=f"I-{nc.next_id()}",
                    act_func_set_id=ACT_SET_LN_EXP,
                    engine=mybir.EngineType.Activation,
                )
            )

            ident = small.tile([HPC, HPC], f32, tag="ident")
            make_identity(nc, ident)
            zeros = small.tile([HPC, SW], f32, tag="zeros")
            nc.gpsimd.memset(zeros, 0.0)

            g = small.tile([HPC, N], f32, tag="g")
            gS16 = small.tile([HPC, N], f16, tag="gS16")
            # tr[p, r, c4, h] = g[h, 512*c4 + 4p + r] (PSUM, via PE transpose)
            tr = trp.tile([P, RPT, NCH4, HPC], f32, tag="tr")
            # ngEO[p, r, c4*HPC + h] = -0.5 * g[h, 512*c4 + 4p + r]
            ngEO = small.tile([P, RPT, NCH4 * HPC], f32, tag="ngEO")
            bcast16 = big.tile([P, HPC, N], f16, tag="bcast16")

            ps_tiles = {}

            def chain_mm(s):
                # (16*logits)^T [4, 512] for segment s, accumulated in PSUM
                ps = lps.tile([HPC, SW], f32, tag="lps")
                ps_tiles[s] = ps
                for c in range(DC):
                    nc.tensor.matmul(
                        ps,
                        Wt_s[:, c, :],
                        xT_s[:, s, c, :],
                        start=(c == 0),
                        stop=(c == DC - 1),
                    )

            def chain_post(s):
                lo, hi = s * SW, (s + 1) * SW
                ps = ps_tiles.pop(s)
                us = usegp.tile([HPC, SW], f32, tag="useg")
                # t = exp(-(logits + b)) = exp(ps * (-1/16) - b)
                nc.scalar.activation(us, ps, Act.Exp, bias=nb[:, 0:1], scale=-1.0 / 16.0)
                # u = ln(1 + t)  (= -logsigmoid(logits), positive)
                nc.scalar.activation(us, us, Act.Ln, bias=1.0)
                # g[:, lo:hi] = cumsum(useg) carried from the previous segment
                init = 0.0 if s == 0 else g[:, lo - 1 : lo]
                nc.vector.tensor_tensor_scan(
                    g[:, lo:hi], us, zeros, init,
                    mybir.AluOpType.add, mybir.AluOpType.add,
                )
                # per-(partition, row-slot) transposes for row chunk c4 = s:
                # tr[:, r, s, :] = g[:, 512s + r :: 4]^T
                for r in range(RPT):
                    nc.tensor.transpose(
                        tr[:, r, s, :], g[:, lo + r : hi : RPT], ident
                    )

            def half_done(half):
                # gS16 half = 0.5*g half (fp16); bounce to DRAM; fan out to
                # all 128 partitions per head via stride-0 reads
                lo = half * NH2
                nc.vector.tensor_scalar_mul(
                    gS16[:, lo : lo + NH2], g[:, lo : lo + NH2], 0.5
                )
                # biases for the two row chunks of this half
                c4lo = half * 2
                nc.vector.tensor_scalar_mul(
                    ngEO[:, :, c4lo * HPC : (c4lo + 2) * HPC],
                    tr[:, :, c4lo : c4lo + 2, :].rearrange(
                        "p r c4 h -> p r (c4 h)"
                    ),
                    -0.5,
                )
                # stage + broadcast DMAs ride the GPSIMD SWDGE ring: their
                # semaphore waits (on the DVE muls) would otherwise block
                # queued output-tile DMAs on the Sync ring (the HWDGE
                # sequencer waits in FIFO order)
                nc.gpsimd.dma_start(
                    out=gdram[:, lo : lo + NH2], in_=gS16[:, lo : lo + NH2]
                )
                for h in range(HPC):
                    nc.gpsimd.dma_start(
                        out=bcast16[:, h, lo : lo + NH2],
                        in_=gdram[h : h + 1, lo : lo + NH2]
                        .partition_broadcast(P)
                        .squeeze(1),
                    )

            # ---- output tiles, elementwise ot = bcast16 + ngEO per row-slot,
            # greedily split between DVE (fast) and ACT; DVE tiles drain on
            # the Sync DMA ring, ACT tiles on the Scalar ring.
            eng_t = {"a": 0.0, "v": 0.0}
            COST = {("a", 0): 4 * 1.04, ("a", 1): 4 * 1.89,
                    ("v", 0): 4 * 0.66, ("v", 1): 4 * 1.19}

            def emit_tile(h, c4, half, force=None):
                # half: 0 = cols 0:1024, 1 = cols 1024:2048, None = full
                full = half is None
                w = N if full else NH2
                lo = 0 if full else half * NH2
                col = c4 * HPC + h
                if force is not None:
                    use_a = force == "a"
                else:
                    use_a = (eng_t["a"] + COST[("a", int(full))]
                             < eng_t["v"] + COST[("v", int(full))])
                eng = "a" if use_a else "v"
                eng_t[eng] += COST[(eng, int(full))]
                pool = outa if use_a else outv
                ot = pool.tile([P, RPT, w], f8e3, tag=f"ot{eng}{int(full)}")
                if full:
                    dst = outC_r[h, c4 - 2]
                else:
                    dst = (outA_r if half == 0 else outB_r)[h, c4]
                dma = (nc.scalar if use_a else nc.sync).dma_start
                for r in range(RPT):
                    if use_a:
                        nc.scalar.activation(
                            ot[:, r, :], bcast16[:, h, lo : lo + w],
                            Act.Identity,
                            bias=ngEO[:, r, col : col + 1], scale=1.0,
                        )
                    else:
                        nc.vector.tensor_scalar_add(
                            ot[:, r, :], bcast16[:, h, lo : lo + w],
                            ngEO[:, r, col : col + 1],
                        )
                dma(out=dst, in_=ot)

            # software-pipelined front; head 0's two left-half tiles slot
            # into the engine gaps while segments 2-3 finish (DVE between
            # scan2 and scan3, ACT right after ln3)
            chain_mm(0)
            chain_mm(1)
            chain_post(0)
            chain_mm(2)
            chain_post(1)
            half_done(0)
            chain_mm(3)
            chain_post(2)
            emit_tile(0, 0, 0, force="v")
            chain_post(3)
            emit_tile(0, 1, 0, force="a")
            half_done(1)

            # phase A: remaining left halves of rows 0:1024; then per head
            # the full-width bottom chunks and right halves (ready after the
            # right bcast)
            for h in range(1, HPC):
                for c4 in (0, 1):
                    emit_tile(h, c4, 0)
            for h in range(HPC):
                emit_tile(h, 2, None)
                emit_tile(h, 3, None)
                emit_tile(h, 0, 1)
                emit_tile(h, 1, 1)

    if not nc.is_finalized():
        nc.finalize()
    return nc


def _get_nc():
    if "nc" not in _CACHE:
        _CACHE["nc"] = _build_nc()
    return _CACHE["nc"]


def _make_in_maps(x, W, b):
    x = np.ascontiguousarray(x, dtype=np.float32)
    W = np.ascontiguousarray(W, dtype=np.float32)
    b = np.ascontiguousarray(b, dtype=np.float32)
    f8e4 = ml_dtypes.float8_e4m3
    # seg-major partition-major: xT[s, p, c, j] = x[bi].T[c*128+p, s*512+j]
    xT_by_batch = [
        np.ascontiguousarray(
            x[bi].T.astype(f8e4)
            .reshape(DC, P, NSEG, SW)
            .transpose(2, 1, 0, 3)
        )
        for bi in range(B)
    ]
    in_maps = []
    for k in range(NCORES):
        bi = k // (NCORES // B)
        h0 = (k % (NCORES // B)) * HPC
        in_maps.append(
            {
                "xT": xT_by_batch[bi],
                "Wt": np.ascontiguousarray(
                    (W[h0 : h0 + HPC].T * 16.0).astype(f8e4)
                ),
                "bv": np.ascontiguousarray(b[h0 : h0 + HPC].reshape(HPC, 1)),
            }
        )
    return in_maps


def kernel(x, W, b, _trace=False, _trace_cores=None):
    from concourse.bass_utils import run_bass_kernel_spmd

    nc = _get_nc()
    in_maps = _make_in_maps(x, W, b)
    res = run_bass_kernel_spmd(
        nc, in_maps, core_ids=list(range(NCORES)), trace=_trace,
        trace_cores=_trace_cores,
    )
    _CACHE["last_results"] = res
    full = np.empty((B, NH, N, N), dtype=np.float32)
    for k in range(NCORES):
        bi = k // (NCORES // B)
        h0 = (k % (NCORES // B)) * HPC
        r = res.results[k]
        # decode e3m4; the 2x undoes the device-side 0.5 scale
        sl = full[bi, h0 : h0 + HPC]
        sl[:, :NH2, :NH2] = r["outA"].astype(np.float32)
        sl[:, :NH2, NH2:] = r["outB"].astype(np.float32)
        sl[:, NH2:, :] = r["outC"].astype(np.float32)
        np.multiply(sl, 2.0, out=sl)
    return full
